# revision 23
# baseline (speedup 1.0000x reference)
"""Bass/Trainium2 kernel for nn_Loss_25546465477236 (YOLO-style detection loss).

Contract: kernel(**inputs) takes FULL unsharded inputs
  pred_tensor  [1024, 80, 80, 5] f32
  target_boxes [1024, 80, 80, 4] f32
  obj_mask     [1024, 80, 80]    i32
and returns the FULL scalar loss (f32), matching the jax reference.

Strategy: pure data parallel over 8 NeuronCores (batch 1024 -> 8 x 128).
Per core, 128 batch items map to the 128 SBUF partitions; the 80*80=6400
cells stream along the free dim in 5 chunks of F=1280.

Host marshaling (layout + dtype narrowing only, no math): the 9 data
planes and the 0/1 mask are packed chunk-major as bf16
  X [N, nchunk, 10, F], planes [px,py,tx,ty,pw,ph,tw,th,pc,m]
(bf16 input quantization is unbiased; measured end-to-end rel err vs the
f32 reference ~2.5e-4, far inside the 2e-2 gate), halving HBM traffic vs
f32. An identity matrix (bf16) rides along for PSUM-diagonal extraction.

Math (validated in numpy against reference.py, rel err 2.5e-4):
  Because the reference's xyxy conversion uses w/S as the center for BOTH
  axes, the x-overlap is EXACTLY min(pw,tw). The y-overlap equals
  relu(min(d+u,v)+min(u-d,v)) with u=ph/2, v=th/2, d=(pw-tw)/80; since
  |d|<=1/80 this is min(ph,th) up to |err|<=2|d| on ~3% of cells, which
  perturbs only the iou term of the loss (~2.6e-4 relative on a term that
  is ~2.6% of the loss) -> approximate ih = min(ph,th).

  All per-cell math runs UNMASKED; the obj mask enters only through the
  reductions (sum of m * plane), computed on the otherwise-idle TensorE
  as 128x128 "diagonal pair" matmuls: stationary = m block, moving =
  plane block, accumulated into a PSUM [128,128] tile whose diagonal
  holds per-partition masked sums; the diagonal is extracted once at the
  end with one fused scalar_tensor_tensor (x identity, accum) per tile.

Engine split (per chunk):
  DVE:    dxy=(px,py)-(tx,ty); u2=(pw,ph)*(tw,th); iwh=min((pw,ph),(tw,th))
          inter=iw*ih; sre=(area_p+eps)+area_t; dnm32=sre-inter (f32);
          r32=reciprocal_approx_fast(dnm32); iou=inter*r32
  GpSimd: areas=[pw*ph|tw*th] (strided pairing); iou_m=iou*m
  ScalarE(one table set, sqrt_and_others; zero table switches):
          dsq=Square(dxy); su2=Sqrt(4*u2)=2*sqrt(u2); psq=Square(pc)+accum
  TensorE: per 128-col block: stationary m -> moving {pw,ph,tw,th,
          dsq.x,dsq.y}->D1 (A12+A3), {su2a,su2b}->D3 (A4), {psq}->D4;
          stationary iou_m -> moving {pc}->D5, {iou}->D6 (A5 terms)

Host combine (f64):  S_k = sum over partitions/cols of tile k
  A12+A3 = S(D1); A4 = S(D3); Sm_psq = S(D4); S_pc_iou = S(D5);
  S_m_iou2 = S(D6); S_psq = sum of per-chunk ScalarE accums
  loss = (5*(S(D1)-S(D3)) + (Sm_psq - 2*S(D5) + S(D6))
          + 0.5*(S_psq - Sm_psq)) / 1024
"""

import numpy as np

import concourse.bass as bass
import concourse.bacc as bacc
import concourse.mybir as mybir
import concourse.tile as tile
from concourse.bass_utils import run_bass_kernel_spmd

N_CORES = 8
B = 1024
PB = B // N_CORES          # 128 batch items per core -> partition dim
CELLS = 80 * 80            # 6400 cells per batch item
F = 1280                   # max cells per chunk (tile sizing)
# small first chunk -> early pipeline start; small last -> short tail
CHW = [512, 1280, 1280, 1280, 1280, 768]
assert sum(CHW) == CELLS and all(w % 128 == 0 for w in CHW)
COFF = [sum(CHW[:i]) for i in range(len(CHW))]
NCHUNK = len(CHW)

f32 = mybir.dt.float32
bf16 = mybir.dt.bfloat16
AL = mybir.AluOpType
AF = mybir.ActivationFunctionType

EPS = 1e-9

# acc layout: cols 0-5 = diag sums of D0..D5; cols 6.. = psq accum per chunk
# D0[512] = m x {pw,ph,tw,th} (A3); D1[256] = m x {dx2,dy2} (A12);
# D2[256] = m x {su2a,su2b} (A4); D3[128] = m x pc^2; D4[128] = ioum x pc;
# D5[128] = ioum x iou
NDIAG = 6
DWID = [4, 2, 2, 1, 1, 1]          # 128-col slabs per diag tile
NACC = NDIAG + NCHUNK


def build_nc(F=F):
    nchunk = NCHUNK
    nc = bacc.Bacc("TRN2", target_bir_lowering=False, debug=False,
                   num_devices=N_CORES)

    x_d = nc.dram_tensor("x", [PB, 10 * CELLS], bf16, kind="ExternalInput")
    id_d = nc.dram_tensor("ident", [PB, 128], bf16, kind="ExternalInput")
    out_d = nc.dram_tensor("acc", [PB, NACC], f32, kind="ExternalOutput")

    with tile.TileContext(nc) as tc:
        with (
            tc.tile_pool(name="io", bufs=3) as io,
            tc.tile_pool(name="wk", bufs=2) as wk,
            tc.tile_pool(name="wk3", bufs=3) as wk3,
            tc.tile_pool(name="acts", bufs=2) as acts,
            tc.tile_pool(name="ps", bufs=1, space="PSUM") as ps,
            tc.tile_pool(name="fix", bufs=1) as fix,
        ):
            acc = fix.tile([PB, NACC], f32, tag="acc")
            ident = fix.tile([PB, 128], bf16, tag="ident")
            dscr = fix.tile([PB, 512], f32, tag="dscr")

            # persistent PSUM diag tiles
            D = [ps.tile([PB, 128 * DWID[k]], f32, tag=f"D{k}", name=f"D{k}")
                 for k in range(NDIAG)]
            first_mm = [True] * NDIAG

            st = [None] * nchunk

            def emit_load(c):
                # split per plane-group so compute starts on first-landed
                # group: wh planes first (most consumers), then xy, then pc+m
                fc = CHW[c]
                base = 10 * COFF[c]
                xc = io.tile([PB, 10 * F], bf16, tag="xc")
                for a, b in ((4, 8), (0, 4), (8, 10)):
                    nc.sync.dma_start(
                        xc[:, a * fc:b * fc],
                        x_d[:, base + a * fc:base + b * fc])
                st[c] = dict(xc=xc)

            def mm(k, mov, stat, fin=False, ldw=True):
                nc.tensor.matmul(D[k][:], stat, mov,
                                 start=first_mm[k], stop=fin)
                first_mm[k] = False

            def emit_stage_a(c):
                """Independent work: DVE front half, DMA-computed min/sum,
                ScalarE activations."""
                s = st[c]
                xc = s["xc"]
                fc = CHW[c]

                def pl(i, j=None):  # flat plane slices (2x-eligible APs)
                    j = i + 1 if j is None else j
                    return xc[:, i * fc:j * fc]

                u2r = wk3.tile([PB, 2 * F], bf16, tag="u2r")
                nc.vector.tensor_tensor(u2r[:, 0:2 * fc], pl(4, 6), pl(6, 8),
                                        AL.mult)
                iwh = wk.tile([PB, 2 * F], bf16, tag="iwh")
                nc.vector.tensor_tensor(iwh[:, 0:2 * fc], pl(4, 6), pl(6, 8),
                                        AL.min)
                area_a = wk.tile([PB, F], bf16, tag="area_a")
                nc.vector.tensor_tensor(area_a[:, 0:fc], pl(4), pl(5), AL.mult)
                area_b = wk.tile([PB, F], bf16, tag="area_b")
                nc.vector.tensor_tensor(area_b[:, 0:fc], pl(6), pl(7), AL.mult)
                # area_a += area_b via DMA CCE -> sum of areas
                nc.gpsimd.dma_start(area_a[:, 0:fc], area_b[:, 0:fc],
                                    accum_op=AL.add)
                dxy = wk3.tile([PB, 2 * F], bf16, tag="dxy")
                nc.vector.tensor_tensor(dxy[:, 0:2 * fc], pl(0, 2), pl(2, 4),
                                        AL.subtract)

                # ---- ScalarE (sqrt_and_others only; no table switches) ----
                su2 = acts.tile([PB, 2 * F], bf16, tag="su2")
                nc.scalar.activation(su2[:, 0:2 * fc], u2r[:, 0:2 * fc],
                                     AF.Sqrt, 0.0, 4.0)
                dsq = acts.tile([PB, 2 * F], bf16, tag="dsq")
                nc.scalar.activation(dsq[:, 0:2 * fc], dxy[:, 0:2 * fc],
                                     AF.Square)
                psq = acts.tile([PB, F], bf16, tag="psq")
                nc.scalar.activation(psq[:, 0:fc], pl(8), AF.Square,
                                     accum_out=acc[:, NDIAG + c:NDIAG + c + 1])
                s.update(xc=xc, iwh=iwh, area_a=area_a,
                         dsq=dsq, su2=su2, psq=psq)

            def emit_stage_b(c):
                """DVE back half: iou chain (waits on stage-A DMAs)."""
                s = st[c]
                xc, iwh, area_a = s["xc"], s["iwh"], s["area_a"]
                fc = CHW[c]
                inter = wk.tile([PB, F], bf16, tag="inter")
                nc.vector.tensor_tensor(inter[:, 0:fc], iwh[:, 0:fc],
                                        iwh[:, fc:2 * fc], AL.mult)
                # dnm32 = (sum_areas + eps) - inter (f32 for reciprocal seed)
                dnm = wk.tile([PB, F], f32, tag="dnm")
                nc.vector.scalar_tensor_tensor(
                    dnm[:, 0:fc], area_a[:, 0:fc], EPS, inter[:, 0:fc],
                    AL.add, AL.subtract)
                r32 = wk.tile([PB, F], f32, tag="r32")
                nc.vector.reciprocal_approx_fast(out=r32[:, 0:fc],
                                                 in_=dnm[:, 0:fc])
                interm = wk.tile([PB, F], bf16, tag="interm")
                nc.vector.tensor_tensor(interm[:, 0:fc], inter[:, 0:fc],
                                        xc[:, 9 * fc:10 * fc], AL.mult)
                ioum = wk3.tile([PB, F], bf16, tag="ioum")
                nc.vector.tensor_tensor(ioum[:, 0:fc], interm[:, 0:fc],
                                        r32[:, 0:fc], AL.mult)
                s.update(ioum=ioum)

            def emit_pairs_d0(c):
                """TensorE wave 0: mask x raw wh planes (ready at DMA land)."""
                s = st[c]
                xc = s["xc"]
                fc = CHW[c]
                xv = xc[:, 0:10 * fc].rearrange("p (n f) -> p n f", n=10)
                last = (c == nchunk - 1)
                for b in range(fc // 128):
                    sl = slice(b * 128, (b + 1) * 128)
                    mm(0, xv[:, 4:8, sl], xv[:, 9, sl],
                       fin=last and (b == fc // 128 - 1))

            def emit_pairs_act(c):
                """TensorE wave 1: ScalarE-produced movings."""
                s = st[c]
                dsq, su2, psq = s["dsq"], s["su2"], s["psq"]
                xc = s["xc"]
                fc = CHW[c]
                xv = xc[:, 0:10 * fc].rearrange("p (n f) -> p n f", n=10)
                last = (c == nchunk - 1)
                d2 = dsq[:, 0:2 * fc].rearrange("p (n f) -> p n f", n=2)
                s2v = su2[:, 0:2 * fc].rearrange("p (n f) -> p n f", n=2)
                for b in range(fc // 128):
                    sl = slice(b * 128, (b + 1) * 128)
                    mblk = xv[:, 9, sl]
                    lastb = last and (b == fc // 128 - 1)
                    mm(1, d2[:, :, sl], mblk, fin=lastb)
                    mm(2, s2v[:, :, sl], mblk, fin=lastb)
                    mm(3, psq[:, sl], mblk, fin=lastb)

            def emit_pairs_late(c):
                """TensorE wave 2: iou-dependent pairs (trail by one chunk)."""
                s = st[c]
                ioum = s["ioum"]
                xc = s["xc"]
                fc = CHW[c]
                xv = xc[:, 0:10 * fc].rearrange("p (n f) -> p n f", n=10)
                last = (c == nchunk - 1)
                for b in range(fc // 128):
                    sl = slice(b * 128, (b + 1) * 128)
                    lastb = last and (b == fc // 128 - 1)
                    mm(4, xv[:, 8, sl], ioum[:, sl], fin=lastb)
                    mm(5, ioum[:, sl], ioum[:, sl], fin=lastb)

            def extract(k):
                # acc[:,k] = sum_f D[k] * tiled-identity (per-partition diag)
                w = DWID[k]
                iv = ident[:].unsqueeze(1).broadcast_to((PB, w, 128))
                dv = D[k][:].rearrange("p (n f) -> p n f", n=w)
                sc = dscr[:, 0:128 * w].rearrange("p (n f) -> p n f", n=w)
                nc.vector.scalar_tensor_tensor(
                    sc, dv, 1.0, iv, AL.mult, AL.mult,
                    accum_out=acc[:, k:k + 1])

            emit_load(0)
            emit_load(1)
            emit_stage_a(0)
            for c in range(nchunk):
                if c + 1 < nchunk:
                    emit_stage_a(c + 1)
                emit_stage_b(c)
                emit_pairs_d0(c)
                emit_pairs_act(c)
                if c > 0:
                    emit_pairs_late(c - 1)
                if c == 0:
                    nc.sync.dma_start(ident[:], id_d[:])
                if c + 2 < nchunk:
                    emit_load(c + 2)
            for k in (0, 1, 2, 3):
                extract(k)
            emit_pairs_late(nchunk - 1)
            for k in (4, 5):
                extract(k)

            nc.sync.dma_start(out_d[:], acc[:])

    nc.compile()
    return nc


_nc_cache = {}


def get_nc(F=F):
    if F not in _nc_cache:
        _nc_cache[F] = build_nc(F)
    return _nc_cache[F]


def make_in_maps(pred_tensor, target_boxes, obj_mask):
    import ml_dtypes
    bf = ml_dtypes.bfloat16
    pred = np.asarray(pred_tensor, dtype=np.float32).reshape(B, CELLS, 5)
    targ = np.asarray(target_boxes, dtype=np.float32).reshape(B, CELLS, 4)
    mask = np.asarray(obj_mask).reshape(B, CELLS)

    planes = (pred[:, :, 0], pred[:, :, 1], targ[:, :, 0], targ[:, :, 1],
              pred[:, :, 2], pred[:, :, 3], targ[:, :, 2], targ[:, :, 3],
              pred[:, :, 4], (mask != 0).astype(np.float32))
    X = np.empty((B, 10 * CELLS), dtype=bf)
    for c, (oc, fc) in enumerate(zip(COFF, CHW)):
        base = 10 * oc
        for i, pl in enumerate(planes):
            X[:, base + i * fc:base + (i + 1) * fc] = \
                pl[:, oc:oc + fc].astype(bf)

    X = X.reshape(N_CORES, PB, 10 * CELLS)
    ident = np.eye(PB, 128, dtype=np.float32).astype(bf)
    return [{"x": X[k], "ident": ident} for k in range(N_CORES)]


def combine_accs(accs):
    """accs: list of per-core [PB, NACC] f32 partials."""
    a = np.asarray(accs, dtype=np.float64)     # [ncores, PB, NACC]
    S = a.sum(axis=(0, 1))                     # [NACC]
    a3, a12, a4, s_mpsq, s_pciou, s_miou2 = S[:NDIAG]
    s_psq = S[NDIAG:].sum()
    loss_sum = (5.0 * (a3 + a12 - a4)
                + (s_mpsq - 2.0 * s_pciou + s_miou2)
                + 0.5 * (s_psq - s_mpsq))
    return np.float32(loss_sum / B)


def kernel(pred_tensor, target_boxes, obj_mask):
    nc = get_nc()
    in_maps = make_in_maps(pred_tensor, target_boxes, obj_mask)
    res = run_bass_kernel_spmd(nc, in_maps, core_ids=list(range(N_CORES)))
    accs = [res.results[k]["acc"] for k in range(N_CORES)]
    return combine_accs(accs)


if __name__ == "__main__":
    rng = np.random.default_rng(0)
    p = rng.random((B, 80, 80, 5), dtype=np.float32)
    t = rng.random((B, 80, 80, 4), dtype=np.float32)
    m = rng.integers(0, 2, size=(B, 80, 80)).astype(np.int32)
    print("loss:", kernel(p, t, m))


# revision 24
# speedup vs baseline: 1.0948x; 1.0948x over previous
"""Bass/Trainium2 kernel for nn_Loss_25546465477236 (YOLO-style detection loss).

Contract: kernel(**inputs) takes FULL unsharded inputs
  pred_tensor  [1024, 80, 80, 5] f32
  target_boxes [1024, 80, 80, 4] f32
  obj_mask     [1024, 80, 80]    i32
and returns the FULL scalar loss (f32), matching the jax reference.

Strategy: pure data parallel over 8 NeuronCores (batch 1024 -> 8 x 128).
Per core, 128 batch items map to the 128 SBUF partitions; the 80*80=6400
cells stream along the free dim in 5 chunks of F=1280.

Host marshaling (layout + dtype narrowing only, no math): the 9 data
planes and the 0/1 mask are packed chunk-major as bf16
  X [N, nchunk, 10, F], planes [px,py,tx,ty,pw,ph,tw,th,pc,m]
(bf16 input quantization is unbiased; measured end-to-end rel err vs the
f32 reference ~2.3e-4, far inside the 2e-2 gate), halving HBM traffic vs
f32. An identity matrix (bf16) rides along for PSUM-diagonal extraction.

Math (validated in numpy against reference.py):
  Because the reference's xyxy conversion uses w/S as the center for BOTH
  axes, the x-overlap is EXACTLY min(pw,tw). The y-overlap equals
  relu(min(d+u,v)+min(u-d,v)) with u=ph/2, v=th/2, d=(pw-tw)/80; since
  |d|<=1/80 this is min(ph,th) up to |err|<=2|d| on ~3% of cells, which
  perturbs only the iou term of the loss (~2.6e-4 relative on a term that
  is ~2.6% of the loss) -> approximate ih = min(ph,th).

  All per-cell math runs UNMASKED; the obj mask enters only through the
  reductions (sum of m * plane), computed on the otherwise-idle TensorE
  as 128x128 "diagonal pair" matmuls: stationary = mask block, moving =
  plane block(s), accumulated into PSUM tiles whose diagonals hold the
  per-partition masked sums; diagonals are extracted at the end with one
  fused scalar_tensor_tensor (x identity, accum) per tile.

Engine split (per chunk) - NOTE GpSimd tensor ops are avoided entirely:
they share an SBUF port with the DVE and measurably serialize with it
(a concurrent GpSimd op slowed DVE ~4x). GpSimd only triggers one small
SBUF->SBUF accumulate DMA (sum of box areas via the DMA CCE adder).
  DVE:     u2=(pw,ph)*(tw,th); iwh=min((pw,ph),(tw,th)); areas; dxy;
           inter=iw*ih; dnm32=(areas+eps)-inter (f32, fused STT);
           r32=reciprocal_approx_fast(dnm32); interm=inter*m;
           ioum=interm*r32  (= m*iou)
  ScalarE (one table set, sqrt_and_others => zero table-load switches):
           su2=Sqrt(4*u2)=2*sqrt(u2); dsq=Square(dxy);
           psq=Square(pc) with fused accum -> sum pc^2
  TensorE waves (software-pipelined; wave 2 trails one chunk so the PE
  never stalls on the iou chain):
    wave0: D0[512] += m x {pw,ph,tw,th}          (A3)
    wave1: D1[256] += m x {dx2,dy2}  (A12); D2[256] += m x {su2a,su2b}
           (A4); D3[128] += m x psq  (Sum m pc^2)
    wave2: D4[128] += ioum x pc; D5[128] += ioum x ioum (A5 terms)

Host combine (f64): S_k = diag-sum of tile k; S_psq = sum of psq accums
  loss = (5*(S0 + S1 - S2) + (S3 - 2*S4 + S5) + 0.5*(S_psq - S3)) / 1024
"""

import numpy as np

import concourse.bass as bass
import concourse.bacc as bacc
import concourse.mybir as mybir
import concourse.tile as tile
from concourse.bass_utils import run_bass_kernel_spmd

N_CORES = 8
B = 1024
PB = B // N_CORES          # 128 batch items per core -> partition dim
CELLS = 80 * 80            # 6400 cells per batch item
F = 1280                   # cells per chunk (divisible by 128)
NCHUNK = CELLS // F
NBLK = F // 128            # diag blocks per chunk

f32 = mybir.dt.float32
bf16 = mybir.dt.bfloat16
AL = mybir.AluOpType
AF = mybir.ActivationFunctionType

EPS = 1e-9

# acc layout: cols 0-5 = diag sums of D0..D5; cols 6.. = psq accum per chunk
NDIAG = 6
DWID = [4, 2, 2, 1, 1, 1]          # 128-col slabs per diag tile
NACC = NDIAG + NCHUNK


def build_nc(F=F):
    nchunk = CELLS // F
    nblk = F // 128
    nc = bacc.Bacc("TRN2", target_bir_lowering=False, debug=False,
                   num_devices=N_CORES)

    x_d = nc.dram_tensor("x", [PB, nchunk * 10 * F], bf16, kind="ExternalInput")
    id_d = nc.dram_tensor("ident", [PB, 128], bf16, kind="ExternalInput")
    out_d = nc.dram_tensor("acc", [PB, NACC], f32, kind="ExternalOutput")

    x4_d = x_d[:].rearrange("p (c n f) -> p c n f", c=nchunk, n=10)

    with tile.TileContext(nc) as tc:
        with (
            tc.tile_pool(name="io", bufs=3) as io,
            tc.tile_pool(name="wk", bufs=2) as wk,
            tc.tile_pool(name="wk3", bufs=3) as wk3,
            tc.tile_pool(name="acts", bufs=2) as acts,
            tc.tile_pool(name="ps", bufs=1, space="PSUM") as ps,
            tc.tile_pool(name="fix", bufs=1) as fix,
        ):
            acc = fix.tile([PB, NACC], f32, tag="acc")
            ident = fix.tile([PB, 128], bf16, tag="ident")
            dscr = fix.tile([PB, 512], f32, tag="dscr")

            # persistent PSUM diag tiles
            D = [ps.tile([PB, 128 * DWID[k]], f32, tag=f"D{k}", name=f"D{k}")
                 for k in range(NDIAG)]
            first_mm = [True] * NDIAG

            st = [None] * nchunk

            def emit_load(c):
                # split per plane-group so compute starts on first-landed
                # group: wh planes first (most consumers), then xy, then pc+m
                xc = io.tile([PB, 10 * F], bf16, tag="xc")
                for a, b in ((4, 8), (0, 4), (8, 10)):
                    nc.sync.dma_start(
                        xc[:, a * F:b * F].rearrange("p (n f) -> p n f",
                                                     n=b - a),
                        x4_d[:, c, a:b, :])
                st[c] = dict(xc=xc)

            def mm(k, mov, stat, fin=False):
                nc.tensor.matmul(D[k][:], stat, mov,
                                 start=first_mm[k], stop=fin)
                first_mm[k] = False

            def emit_stage_a(c):
                """Independent work: DVE front half, DMA-computed area sum,
                ScalarE activations."""
                s = st[c]
                xc = s["xc"]
                xv = xc[:].rearrange("p (n f) -> p n f", n=10)

                def pl(i, j=None):  # flat plane slices (2x-eligible APs)
                    j = i + 1 if j is None else j
                    return xc[:, i * F:j * F]

                u2r = wk3.tile([PB, 2 * F], bf16, tag="u2r")
                nc.vector.tensor_tensor(u2r[:], pl(4, 6), pl(6, 8), AL.mult)
                iwh = wk.tile([PB, 2 * F], bf16, tag="iwh")
                nc.vector.tensor_tensor(iwh[:], pl(4, 6), pl(6, 8), AL.min)
                area_a = wk.tile([PB, F], bf16, tag="area_a")
                nc.vector.tensor_tensor(area_a[:], pl(4), pl(5), AL.mult)
                area_b = wk.tile([PB, F], bf16, tag="area_b")
                nc.vector.tensor_tensor(area_b[:], pl(6), pl(7), AL.mult)
                # area_a += area_b via DMA CCE -> sum of areas
                nc.gpsimd.dma_start(area_a[:], area_b[:], accum_op=AL.add)
                dxy = wk3.tile([PB, 2 * F], bf16, tag="dxy")
                nc.vector.tensor_tensor(dxy[:], pl(0, 2), pl(2, 4), AL.subtract)

                # ---- ScalarE (sqrt_and_others only; no table switches) ----
                su2 = acts.tile([PB, 2 * F], bf16, tag="su2")
                nc.scalar.activation(su2[:], u2r[:], AF.Sqrt, 0.0, 4.0)
                dsq = acts.tile([PB, 2 * F], bf16, tag="dsq")
                nc.scalar.activation(dsq[:], dxy[:], AF.Square)
                psq = acts.tile([PB, F], bf16, tag="psq")
                nc.scalar.activation(psq[:], xv[:, 8, :], AF.Square,
                                     accum_out=acc[:, NDIAG + c:NDIAG + c + 1])
                s.update(xv=xv, iwh=iwh, area_a=area_a,
                         dsq=dsq, su2=su2, psq=psq)

            def emit_stage_b(c):
                """DVE back half: iou chain (waits on stage-A area DMA)."""
                s = st[c]
                xc, iwh, area_a = s["xc"], s["iwh"], s["area_a"]
                inter = wk.tile([PB, F], bf16, tag="inter")
                nc.vector.tensor_tensor(inter[:], iwh[:, 0:F], iwh[:, F:2 * F],
                                        AL.mult)
                # dnm32 = (sum_areas + eps) - inter (f32 for reciprocal seed)
                dnm = wk.tile([PB, F], f32, tag="dnm")
                nc.vector.scalar_tensor_tensor(
                    dnm[:], area_a[:], EPS, inter[:], AL.add, AL.subtract)
                r32 = wk.tile([PB, F], f32, tag="r32")
                nc.vector.reciprocal_approx_fast(out=r32[:], in_=dnm[:])
                interm = wk.tile([PB, F], bf16, tag="interm")
                nc.vector.tensor_tensor(interm[:], inter[:],
                                        xc[:, 9 * F:10 * F], AL.mult)
                ioum = wk3.tile([PB, F], bf16, tag="ioum")
                nc.vector.tensor_tensor(ioum[:], interm[:], r32[:], AL.mult)
                s.update(ioum=ioum)

            def emit_pairs_d0(c):
                """TensorE wave 0: mask x raw wh planes (ready at DMA land)."""
                s = st[c]
                xv = s["xv"]
                last = (c == nchunk - 1)
                for b in range(nblk):
                    sl = slice(b * 128, (b + 1) * 128)
                    mm(0, xv[:, 4:8, sl], xv[:, 9, sl],
                       fin=last and (b == nblk - 1))

            def emit_pairs_act(c):
                """TensorE wave 1: ScalarE-produced movings."""
                s = st[c]
                xv, dsq, su2, psq = s["xv"], s["dsq"], s["su2"], s["psq"]
                last = (c == nchunk - 1)
                d2 = dsq[:].rearrange("p (n f) -> p n f", n=2)
                s2v = su2[:].rearrange("p (n f) -> p n f", n=2)
                for b in range(nblk):
                    sl = slice(b * 128, (b + 1) * 128)
                    mblk = xv[:, 9, sl]
                    lastb = last and (b == nblk - 1)
                    mm(1, d2[:, :, sl], mblk, fin=lastb)
                    mm(2, s2v[:, :, sl], mblk, fin=lastb)
                    mm(3, psq[:, sl], mblk, fin=lastb)

            def emit_pairs_late(c):
                """TensorE wave 2: iou-dependent pairs (trail by one chunk)."""
                s = st[c]
                xv, ioum = s["xv"], s["ioum"]
                last = (c == nchunk - 1)
                for b in range(nblk):
                    sl = slice(b * 128, (b + 1) * 128)
                    lastb = last and (b == nblk - 1)
                    mm(4, xv[:, 8, sl], ioum[:, sl], fin=lastb)
                    mm(5, ioum[:, sl], ioum[:, sl], fin=lastb)

            def extract(k):
                # acc[:,k] = sum_f D[k] * tiled-identity (per-partition diag)
                w = DWID[k]
                iv = ident[:].unsqueeze(1).broadcast_to((PB, w, 128))
                dv = D[k][:].rearrange("p (n f) -> p n f", n=w)
                sc = dscr[:, 0:128 * w].rearrange("p (n f) -> p n f", n=w)
                nc.vector.scalar_tensor_tensor(
                    sc, dv, 1.0, iv, AL.mult, AL.mult,
                    accum_out=acc[:, k:k + 1])

            emit_load(0)
            emit_load(1)
            emit_stage_a(0)
            for c in range(nchunk):
                if c + 1 < nchunk:
                    emit_stage_a(c + 1)
                emit_stage_b(c)
                emit_pairs_d0(c)
                emit_pairs_act(c)
                if c > 0:
                    emit_pairs_late(c - 1)
                if c == 0:
                    nc.sync.dma_start(ident[:], id_d[:])
                if c + 2 < nchunk:
                    emit_load(c + 2)
            for k in (0, 1, 2, 3):
                extract(k)
            emit_pairs_late(nchunk - 1)
            for k in (4, 5):
                extract(k)

            nc.sync.dma_start(out_d[:], acc[:])

    nc.compile()
    return nc


_nc_cache = {}


def get_nc(F=F):
    if F not in _nc_cache:
        _nc_cache[F] = build_nc(F)
    return _nc_cache[F]


def make_in_maps(pred_tensor, target_boxes, obj_mask):
    import ml_dtypes
    bf = ml_dtypes.bfloat16
    pred = np.asarray(pred_tensor, dtype=np.float32).reshape(B, CELLS, 5)
    targ = np.asarray(target_boxes, dtype=np.float32).reshape(B, CELLS, 4)
    mask = np.asarray(obj_mask).reshape(B, CELLS)

    X = np.empty((B, NCHUNK, 10, F), dtype=bf)
    planes = (pred[:, :, 0], pred[:, :, 1], targ[:, :, 0], targ[:, :, 1],
              pred[:, :, 2], pred[:, :, 3], targ[:, :, 2], targ[:, :, 3],
              pred[:, :, 4], (mask != 0).astype(np.float32))
    for i, pl in enumerate(planes):
        X[:, :, i, :] = pl.reshape(B, NCHUNK, F).astype(bf)

    X = X.reshape(N_CORES, PB, NCHUNK * 10 * F)
    ident = np.eye(PB, 128, dtype=np.float32).astype(bf)
    return [{"x": X[k], "ident": ident} for k in range(N_CORES)]


def combine_accs(accs):
    """accs: list of per-core [PB, NACC] f32 partials."""
    a = np.asarray(accs, dtype=np.float64)     # [ncores, PB, NACC]
    S = a.sum(axis=(0, 1))                     # [NACC]
    a3, a12, a4, s_mpsq, s_pciou, s_miou2 = S[:NDIAG]
    s_psq = S[NDIAG:].sum()
    loss_sum = (5.0 * (a3 + a12 - a4)
                + (s_mpsq - 2.0 * s_pciou + s_miou2)
                + 0.5 * (s_psq - s_mpsq))
    return np.float32(loss_sum / B)


def kernel(pred_tensor, target_boxes, obj_mask):
    nc = get_nc()
    in_maps = make_in_maps(pred_tensor, target_boxes, obj_mask)
    res = run_bass_kernel_spmd(nc, in_maps, core_ids=list(range(N_CORES)))
    accs = [res.results[k]["acc"] for k in range(N_CORES)]
    return combine_accs(accs)


if __name__ == "__main__":
    rng = np.random.default_rng(0)
    p = rng.random((B, 80, 80, 5), dtype=np.float32)
    t = rng.random((B, 80, 80, 4), dtype=np.float32)
    m = rng.integers(0, 2, size=(B, 80, 80)).astype(np.int32)
    print("loss:", kernel(p, t, m))


# revision 25
# speedup vs baseline: 1.2019x; 1.0978x over previous
"""Bass/Trainium2 kernel for nn_Loss_25546465477236 (YOLO-style detection loss).

Contract: kernel(**inputs) takes FULL unsharded inputs
  pred_tensor  [1024, 80, 80, 5] f32
  target_boxes [1024, 80, 80, 4] f32
  obj_mask     [1024, 80, 80]    i32
and returns the FULL scalar loss (f32), matching the jax reference.

Strategy: pure data parallel over 8 NeuronCores (batch 1024 -> 8 x 128).
Per core, 128 batch items map to the 128 SBUF partitions; the 80*80=6400
cells stream along the free dim in 5 chunks of F=1280.

Host marshaling (layout + dtype narrowing only, no math): the 9 data
planes and the 0/1 mask are packed chunk-major as bf16
  X [N, nchunk, 10, F], planes [px,py,tx,ty,pw,ph,tw,th,pc,m]
(bf16 input quantization is unbiased; measured end-to-end rel err vs the
f32 reference ~2.3e-4, far inside the 2e-2 gate), halving HBM traffic vs
f32. An identity matrix (bf16) rides along for PSUM-diagonal extraction.

Math (validated in numpy against reference.py):
  Because the reference's xyxy conversion uses w/S as the center for BOTH
  axes, the x-overlap is EXACTLY min(pw,tw). The y-overlap equals
  relu(min(d+u,v)+min(u-d,v)) with u=ph/2, v=th/2, d=(pw-tw)/80; since
  |d|<=1/80 this is min(ph,th) up to |err|<=2|d| on ~3% of cells, which
  perturbs only the iou term of the loss (~2.6e-4 relative on a term that
  is ~2.6% of the loss) -> approximate ih = min(ph,th).

  All per-cell math runs UNMASKED; the obj mask enters only through the
  reductions (sum of m * plane), computed on the otherwise-idle TensorE
  as 128x128 "diagonal pair" matmuls: stationary = mask block, moving =
  plane block(s), accumulated into PSUM tiles whose diagonals hold the
  per-partition masked sums; diagonals are extracted at the end with one
  fused scalar_tensor_tensor (x identity, accum) per tile.

Engine split (per chunk) - NOTE GpSimd tensor ops are avoided entirely:
they share an SBUF port with the DVE and measurably serialize with it
(a concurrent GpSimd op slowed DVE ~4x). GpSimd only triggers one small
SBUF->SBUF accumulate DMA (sum of box areas via the DMA CCE adder).
  DVE:     u2=(pw,ph)*(tw,th); iwh=min((pw,ph),(tw,th)); areas; dxy;
           inter=iw*ih; dnm32=(areas+eps)-inter (f32, fused STT);
           r32=reciprocal_approx_fast(dnm32); interm=inter*m;
           ioum=interm*r32  (= m*iou)
  ScalarE (one table set, sqrt_and_others => zero table-load switches):
           su2=Sqrt(4*u2)=2*sqrt(u2); dsq=Square(dxy);
           psq=Square(pc) with fused accum -> sum pc^2
  TensorE waves (software-pipelined; wave 2 trails one chunk so the PE
  never stalls on the iou chain):
    wave0: D0[512] += m x {pw,ph,tw,th}          (A3)
    wave1: D1[256] += m x {dx2,dy2}  (A12); D2[256] += m x {su2a,su2b}
           (A4); D3[128] += m x psq  (Sum m pc^2)
    wave2: D4[128] += ioum x pc; D5[128] += ioum x ioum (A5 terms)

Host combine (f64): S_k = diag-sum of tile k; S_psq = sum of psq accums
  loss = (5*(S0 + S1 - S2) + (S3 - 2*S4 + S5) + 0.5*(S_psq - S3)) / 1024
"""

import numpy as np

import concourse.bass as bass
import concourse.bacc as bacc
import concourse.mybir as mybir
import concourse.tile as tile
from concourse.bass_utils import run_bass_kernel_spmd

N_CORES = 8
B = 1024
PB = B // N_CORES          # 128 batch items per core -> partition dim
CELLS = 80 * 80            # 6400 cells per batch item
F = 1280                   # cells per chunk (divisible by 128)
NCHUNK = CELLS // F
NBLK = F // 128            # diag blocks per chunk

f32 = mybir.dt.float32
bf16 = mybir.dt.bfloat16
AL = mybir.AluOpType
AF = mybir.ActivationFunctionType

EPS = 1e-9

# acc layout: cols 0-5 = diag sums of D0..D5; cols 6.. = psq accum per chunk
NDIAG = 6
DWID = [4, 2, 2, 1, 1, 1]          # 128-col slabs per diag tile
NACC = NDIAG + NCHUNK


def build_nc(F=F):
    nchunk = CELLS // F
    nblk = F // 128
    nc = bacc.Bacc("TRN2", target_bir_lowering=False, debug=False,
                   num_devices=N_CORES)

    x_d = nc.dram_tensor("x", [PB, nchunk * 10 * F], bf16, kind="ExternalInput")
    id_d = nc.dram_tensor("ident", [PB, 128], bf16, kind="ExternalInput")
    out_d = nc.dram_tensor("acc", [PB, NACC], f32, kind="ExternalOutput")

    x4_d = x_d[:].rearrange("p (c n f) -> p c n f", c=nchunk, n=10)

    with tile.TileContext(nc) as tc:
        with (
            tc.tile_pool(name="io", bufs=3) as io,
            tc.tile_pool(name="wk", bufs=2) as wk,
            tc.tile_pool(name="wk3", bufs=3) as wk3,
            tc.tile_pool(name="acts", bufs=2) as acts,
            tc.tile_pool(name="ps", bufs=1, space="PSUM") as ps,
            tc.tile_pool(name="fix", bufs=1) as fix,
        ):
            acc = fix.tile([PB, NACC], f32, tag="acc")
            ident = fix.tile([PB, 128], bf16, tag="ident")
            dscr = fix.tile([PB, 512], f32, tag="dscr")
            nc.sync.dma_start(ident[:], id_d[:])

            # persistent PSUM diag tiles
            D = [ps.tile([PB, 128 * DWID[k]], f32, tag=f"D{k}", name=f"D{k}")
                 for k in range(NDIAG)]
            first_mm = [True] * NDIAG

            st = [None] * nchunk

            def emit_load(c):
                # split per plane-group so compute starts on first-landed
                # group: wh planes first (most consumers), then xy, then pc+m
                xc = io.tile([PB, 10 * F], bf16, tag="xc")
                for a, b in ((4, 8), (0, 4), (8, 10)):
                    nc.sync.dma_start(
                        xc[:, a * F:b * F].rearrange("p (n f) -> p n f",
                                                     n=b - a),
                        x4_d[:, c, a:b, :])
                st[c] = dict(xc=xc)

            def mm(k, mov, stat, fin=False):
                nc.tensor.matmul(D[k][:], stat, mov,
                                 start=first_mm[k], stop=fin)
                first_mm[k] = False

            def emit_stage_a(c):
                """Independent work: DVE front half, DMA-computed area sum,
                ScalarE activations."""
                s = st[c]
                xc = s["xc"]
                xv = xc[:].rearrange("p (n f) -> p n f", n=10)

                def pl(i, j=None):  # flat plane slices (2x-eligible APs)
                    j = i + 1 if j is None else j
                    return xc[:, i * F:j * F]

                u2r = wk3.tile([PB, 2 * F], bf16, tag="u2r")
                nc.vector.tensor_tensor(u2r[:], pl(4, 6), pl(6, 8), AL.mult)
                iwh = wk.tile([PB, 2 * F], bf16, tag="iwh")
                nc.vector.tensor_tensor(iwh[:], pl(4, 6), pl(6, 8), AL.min)
                area_a = wk.tile([PB, F], bf16, tag="area_a")
                nc.vector.tensor_tensor(area_a[:], pl(4), pl(5), AL.mult)
                area_b = wk.tile([PB, F], bf16, tag="area_b")
                nc.vector.tensor_tensor(area_b[:], pl(6), pl(7), AL.mult)
                # area_a += area_b via DMA CCE -> sum of areas
                nc.gpsimd.dma_start(area_a[:], area_b[:], accum_op=AL.add)
                dxy = wk3.tile([PB, 2 * F], bf16, tag="dxy")
                nc.vector.tensor_tensor(dxy[:], pl(0, 2), pl(2, 4), AL.subtract)

                # ---- ScalarE (sqrt_and_others only; no table switches) ----
                su2 = acts.tile([PB, 2 * F], bf16, tag="su2")
                nc.scalar.activation(su2[:], u2r[:], AF.Sqrt, 0.0, 4.0)
                dsq = acts.tile([PB, 2 * F], bf16, tag="dsq")
                nc.scalar.activation(dsq[:], dxy[:], AF.Square)
                psq = acts.tile([PB, F], bf16, tag="psq")
                nc.scalar.activation(psq[:], xv[:, 8, :], AF.Square,
                                     accum_out=acc[:, NDIAG + c:NDIAG + c + 1])
                s.update(xv=xv, iwh=iwh, area_a=area_a,
                         dsq=dsq, su2=su2, psq=psq)

            def emit_stage_b(c):
                """DVE back half: iou chain (waits on stage-A area DMA)."""
                s = st[c]
                xc, iwh, area_a = s["xc"], s["iwh"], s["area_a"]
                inter = wk.tile([PB, F], bf16, tag="inter")
                nc.vector.tensor_tensor(inter[:], iwh[:, 0:F], iwh[:, F:2 * F],
                                        AL.mult)
                # dnm32 = (sum_areas + eps) - inter (f32 for reciprocal seed)
                dnm = wk.tile([PB, F], f32, tag="dnm")
                nc.vector.scalar_tensor_tensor(
                    dnm[:], area_a[:], EPS, inter[:], AL.add, AL.subtract)
                r32 = wk.tile([PB, F], f32, tag="r32")
                nc.vector.reciprocal_approx_fast(out=r32[:], in_=dnm[:])
                interm = wk.tile([PB, F], bf16, tag="interm")
                nc.vector.tensor_tensor(interm[:], inter[:],
                                        xc[:, 9 * F:10 * F], AL.mult)
                ioum = wk3.tile([PB, F], bf16, tag="ioum")
                nc.vector.tensor_tensor(ioum[:], interm[:], r32[:], AL.mult)
                s.update(ioum=ioum)

            def emit_pairs_d0(c):
                """TensorE wave 0: mask x raw wh planes (ready at DMA land)."""
                s = st[c]
                xv = s["xv"]
                last = (c == nchunk - 1)
                for b in range(nblk):
                    sl = slice(b * 128, (b + 1) * 128)
                    mm(0, xv[:, 4:8, sl], xv[:, 9, sl],
                       fin=last and (b == nblk - 1))

            def emit_pairs_act(c):
                """TensorE wave 1: ScalarE-produced movings."""
                s = st[c]
                xv, dsq, su2, psq = s["xv"], s["dsq"], s["su2"], s["psq"]
                last = (c == nchunk - 1)
                d2 = dsq[:].rearrange("p (n f) -> p n f", n=2)
                s2v = su2[:].rearrange("p (n f) -> p n f", n=2)
                for b in range(nblk):
                    sl = slice(b * 128, (b + 1) * 128)
                    mblk = xv[:, 9, sl]
                    lastb = last and (b == nblk - 1)
                    mm(1, d2[:, :, sl], mblk, fin=lastb)
                    mm(2, s2v[:, :, sl], mblk, fin=lastb)
                    mm(3, psq[:, sl], mblk, fin=lastb)

            def emit_pairs_late(c):
                """TensorE wave 2: iou-dependent pairs (trail by one chunk)."""
                s = st[c]
                xv, ioum = s["xv"], s["ioum"]
                last = (c == nchunk - 1)
                for b in range(nblk):
                    sl = slice(b * 128, (b + 1) * 128)
                    lastb = last and (b == nblk - 1)
                    mm(4, xv[:, 8, sl], ioum[:, sl], fin=lastb)
                    mm(5, ioum[:, sl], ioum[:, sl], fin=lastb)

            def extract(k):
                # acc[:,k] = sum_f D[k] * tiled-identity (per-partition diag)
                w = DWID[k]
                iv = ident[:].unsqueeze(1).broadcast_to((PB, w, 128))
                dv = D[k][:].rearrange("p (n f) -> p n f", n=w)
                sc = dscr[:, 0:128 * w].rearrange("p (n f) -> p n f", n=w)
                nc.vector.scalar_tensor_tensor(
                    sc, dv, 1.0, iv, AL.mult, AL.mult,
                    accum_out=acc[:, k:k + 1])

            emit_load(0)
            emit_load(1)
            emit_stage_a(0)
            for c in range(nchunk):
                if c + 1 < nchunk:
                    emit_stage_a(c + 1)
                emit_stage_b(c)
                emit_pairs_d0(c)
                emit_pairs_act(c)
                if c > 0:
                    emit_pairs_late(c - 1)
                if c + 2 < nchunk:
                    emit_load(c + 2)
            emit_pairs_late(nchunk - 1)
            for k in range(NDIAG):
                extract(k)

            nc.sync.dma_start(out_d[:], acc[:])

    nc.compile()
    return nc


_nc_cache = {}


def get_nc(F=F):
    if F not in _nc_cache:
        _nc_cache[F] = build_nc(F)
    return _nc_cache[F]


def make_in_maps(pred_tensor, target_boxes, obj_mask):
    import ml_dtypes
    bf = ml_dtypes.bfloat16
    pred = np.asarray(pred_tensor, dtype=np.float32).reshape(B, CELLS, 5)
    targ = np.asarray(target_boxes, dtype=np.float32).reshape(B, CELLS, 4)
    mask = np.asarray(obj_mask).reshape(B, CELLS)

    X = np.empty((B, NCHUNK, 10, F), dtype=bf)
    planes = (pred[:, :, 0], pred[:, :, 1], targ[:, :, 0], targ[:, :, 1],
              pred[:, :, 2], pred[:, :, 3], targ[:, :, 2], targ[:, :, 3],
              pred[:, :, 4], (mask != 0).astype(np.float32))
    for i, pl in enumerate(planes):
        X[:, :, i, :] = pl.reshape(B, NCHUNK, F).astype(bf)

    X = X.reshape(N_CORES, PB, NCHUNK * 10 * F)
    ident = np.eye(PB, 128, dtype=np.float32).astype(bf)
    return [{"x": X[k], "ident": ident} for k in range(N_CORES)]


def combine_accs(accs):
    """accs: list of per-core [PB, NACC] f32 partials."""
    a = np.asarray(accs, dtype=np.float64)     # [ncores, PB, NACC]
    S = a.sum(axis=(0, 1))                     # [NACC]
    a3, a12, a4, s_mpsq, s_pciou, s_miou2 = S[:NDIAG]
    s_psq = S[NDIAG:].sum()
    loss_sum = (5.0 * (a3 + a12 - a4)
                + (s_mpsq - 2.0 * s_pciou + s_miou2)
                + 0.5 * (s_psq - s_mpsq))
    return np.float32(loss_sum / B)


def kernel(pred_tensor, target_boxes, obj_mask):
    nc = get_nc()
    in_maps = make_in_maps(pred_tensor, target_boxes, obj_mask)
    res = run_bass_kernel_spmd(nc, in_maps, core_ids=list(range(N_CORES)))
    accs = [res.results[k]["acc"] for k in range(N_CORES)]
    return combine_accs(accs)


if __name__ == "__main__":
    rng = np.random.default_rng(0)
    p = rng.random((B, 80, 80, 5), dtype=np.float32)
    t = rng.random((B, 80, 80, 4), dtype=np.float32)
    m = rng.integers(0, 2, size=(B, 80, 80)).astype(np.int32)
    print("loss:", kernel(p, t, m))
